# revision 1
# baseline (speedup 1.0000x reference)
"""Trainium2 Bass kernel for nn_MultiHeadAttention_7584912245188.

Reference computes (no softmax!):
    qkv = x @ Wqkv + bqkv ; split q,k,v ; per head: y = (q k^T / sqrt(D)) v
    out = y @ Wff + bff

Because there is no softmax, attention is linear and reassociates:
    (Q K^T) V = Q (K^T V).
With X_aug = [X | 1] ([N, 97]) and G = X_aug^T X_aug ([97, 97]), the whole
module collapses (associativity, per head h):
    out = X_aug @ Wfin,   Wfin = sum_h P_h G Q_h + e_last bff^T
    P_h = Wq_aug_h Wk_aug_h^T [97,97],  Q_h = D^-0.5 Wv_aug_h Wff_h [97,96]
P_h / Q_h are host-precomputed from the weights. On device per batch:
    G (16 accumulating matmuls over row chunks)
    R = G @ [Q_0|...|Q_5]            (2 matmuls, free dim 288)
    Wfin = sum_h P_h R_h + bff term  (7 PSUM-accumulating matmuls, no hops)
    out chunks = X_chunk @ Wfin      (via PE-transposed X chunks)
O(N*E^2) instead of O(N^2*D).

Precision: x is cast to fp16 on the host (halves HBM load traffic); all
matmul operands are fp16 (exact products, f32 PSUM accumulate); Gram
partials accumulate in f32; the output is written fp16 and upcast on the
host. Measured end-to-end rel err ~6e-4.

Sharding (8 cores): core c -> (batch b = c//2, sequence half h = c%2).
Each core receives x[b] (with a ones column appended host-side) rolled so
"its" half comes first, computes G from the full batch (redundantly within
the pair - cheaper than a collective), and writes only its half of the
output rows.

Row layout: x is processed as 2 halves of 1024 rows; within a half,
row = 1024*h + 8*p + j (p = SBUF partition, j = 0..7), so each half is
ONE fully contiguous ~1.5KB segment per partition per DMA (the cost is
dominated by descriptor count, not bytes). Chunks c = 8*h + j are
arbitrary row groups: G sums over all rows regardless of grouping, and
the transpose/final/output steps use the same mapping consistently.

DMA plan: HWDGE DMAs serialize on shared HW lanes (~0.6us slot per
~100KB chunk) and each DMA pays ~0.9us completion latency, so x goes as
4 quarter DMAs (own tile each - readers wait only on their quarter) on
the two HWDGE rings, the packed weights ride the separate SWDGE path,
and the Gram matmuls pipeline behind the quarter arrivals. The transpose
identity is built on-chip by gpsimd before the weights DMA.
"""

import numpy as np
from contextlib import ExitStack

import concourse.bass as bass
import concourse.tile as tile
from concourse import bacc, mybir
from concourse import bass_utils
from concourse.masks import make_identity

B, N, E = 4, 2048, 96
H = 6
D = E // H            # 16
P = 128
NCH = N // P          # 16 chunks of 128 rows
HALF = NCH // 2       # 8 chunks per core
J = 4                 # rows per partition per quarter
EA = E + 1            # 97 (augmented with ones column)
SCALE = float(D) ** -0.5
F32 = mybir.dt.float32
F16 = mybir.dt.float16

# wpack (fp16, 97 partitions) column layout: PcatT | Qcat | onehot | bff
C_P = 0
C_Q = C_P + H * EA           # 582
C_OH = C_Q + H * E           # 1158
C_BF = C_OH + EA             # 1255
WPACK_COLS = C_BF + E        # 1351

N_CORES = 8

_NC_CACHE = {}
LAST_RESULTS = None


def _build_nc():
    nc = bacc.Bacc(
        "TRN2", target_bir_lowering=False, debug=False, num_devices=N_CORES
    )
    x = nc.dram_tensor("x", [N, EA], F16, kind="ExternalInput").ap()
    wpacki = nc.dram_tensor("wpack", [EA, WPACK_COLS], F16, kind="ExternalInput").ap()
    out = nc.dram_tensor("out", [N // 2, E], F16, kind="ExternalOutput").ap()

    with tile.TileContext(nc) as tc, ExitStack() as ctx:
        consts = ctx.enter_context(tc.tile_pool(name="consts", bufs=1))
        big = ctx.enter_context(tc.tile_pool(name="big", bufs=1))
        small = ctx.enter_context(tc.tile_pool(name="small", bufs=1))
        outp = ctx.enter_context(tc.tile_pool(name="outp", bufs=1))
        ps_t = ctx.enter_context(tc.tile_pool(name="ps_t", bufs=2, space="PSUM"))
        ps_g = ctx.enter_context(tc.tile_pool(name="ps_g", bufs=1, space="PSUM"))
        ps_r = ctx.enter_context(tc.tile_pool(name="ps_r", bufs=2, space="PSUM"))
        ps_w = ctx.enter_context(tc.tile_pool(name="ps_w", bufs=1, space="PSUM"))
        ps_o = ctx.enter_context(tc.tile_pool(name="ps_o", bufs=2, space="PSUM"))

        # --- loads: identity+weights via gpsimd/SWDGE (own path), x as two
        # half DMAs on the two HWDGE rings (one tile per DMA)
        xh = x.rearrange("(h p j) e -> h p j e", h=2, j=HALF)
        XA = big.tile([P, HALF, EA], F16)
        nc.sync.dma_start(out=XA[:], in_=xh[0])
        XB = big.tile([P, HALF, EA], F16)
        nc.gpsimd.dma_start(out=XB[:], in_=xh[1])                 # SWDGE
        wp = consts.tile([EA, WPACK_COLS], F16)
        nc.gpsimd.dma_start(out=wp[:], in_=wpacki)                # SWDGE
        id_sb = consts.tile([P, P], F16)
        make_identity(nc, id_sb[:])                               # gpsimd


        def Xc(c):
            return XA[:, c, :] if c < HALF else XB[:, c - HALF, :]

        # --- G = X_aug^T X_aug: one 16-matmul PSUM accumulation group,
        # half B pipelining behind its DMA
        g_ps = ps_g.tile([EA, EA], F32)
        for c in range(NCH):
            nc.tensor.matmul(
                g_ps[:], lhsT=Xc(c), rhs=Xc(c),
                start=(c == 0), stop=(c == NCH - 1),
            )
        g_h = small.tile([EA, EA], F16)
        nc.vector.tensor_copy(out=g_h[:], in_=g_ps[:])

        # --- chain with transposes interleaved into its latency gaps.
        # R = G @ Qcat (2 matmuls, free 288) staged to fp16; the 8 PE
        # transposes of my half run while DVE casts G / stages R.
        XT = big.tile([EA, HALF, P], F16)
        r_h = small.tile([EA, H * E], F16)
        pt0 = ps_t.tile([EA, J, P], F16, tag="pt", name="pt0")
        for j in range(J):
            nc.tensor.transpose(out=pt0[:, j, :], in_=Xc(j), identity=id_sb[:])
        nc.vector.tensor_copy(out=XT[:, 0:J, :], in_=pt0[:])
        r0 = ps_r.tile([EA, H * E // 2], F32, tag="r", name="r0")
        nc.tensor.matmul(
            r0[:], lhsT=g_h[:], rhs=wp[:, C_Q : C_Q + H * E // 2],
            start=True, stop=True,
        )
        nc.vector.tensor_copy(out=r_h[:, 0 : H * E // 2], in_=r0[:])
        pt1 = ps_t.tile([EA, J, P], F16, tag="pt", name="pt1")
        for j in range(J):
            nc.tensor.transpose(
                out=pt1[:, j, :], in_=Xc(J + j), identity=id_sb[:]
            )
        nc.vector.tensor_copy(out=XT[:, J : 2 * J, :], in_=pt1[:])
        r1 = ps_r.tile([EA, H * E // 2], F32, tag="r", name="r1")
        nc.tensor.matmul(
            r1[:], lhsT=g_h[:], rhs=wp[:, C_Q + H * E // 2 : C_Q + H * E],
            start=True, stop=True,
        )
        nc.vector.tensor_copy(out=r_h[:, H * E // 2 : H * E], in_=r1[:])

        # --- Wfin = sum_h P_h R_h + e_last bff^T  (one PSUM accum group)
        wf_ps = ps_w.tile([EA, E], F32)
        for h in range(H):
            nc.tensor.matmul(
                wf_ps[:],
                lhsT=wp[:, C_P + h * EA : C_P + (h + 1) * EA],
                rhs=r_h[:, h * E : (h + 1) * E],
                start=(h == 0),
                stop=False,
            )
        nc.tensor.matmul(
            wf_ps[:],
            lhsT=wp[0:1, C_OH : C_OH + EA],
            rhs=wp[0:1, C_BF : C_BF + E],
            start=False,
            stop=True,
        )
        wf_h = small.tile([EA, E], F16)
        nc.vector.tensor_copy(out=wf_h[:], in_=wf_ps[:])

        # --- finals: out chunk = X_chunk @ Wfin via lhsT = XT chunk
        osb = outp.tile([P, HALF, E], F16)
        for grp in range(2):
            og = ps_o.tile([P, J, E], F32, tag="og", name=f"og{grp}")
            for j in range(J):
                nc.tensor.matmul(
                    og[:, j, :], lhsT=XT[:, J * grp + j, :], rhs=wf_h[:],
                    start=True, stop=True,
                )
            nc.vector.tensor_copy(
                out=osb[:, J * grp : J * (grp + 1), :], in_=og[:]
            )
        nc.sync.dma_start(
            out=out.rearrange("(p j) e -> p j e", j=HALF), in_=osb[:]
        )

    nc.compile()
    return nc


def get_nc():
    if "nc" not in _NC_CACHE:
        _NC_CACHE["nc"] = _build_nc()
    return _NC_CACHE["nc"]


def _host_weights(Wqkv, bqkv, Wff, bff):
    waug = np.concatenate(
        [np.asarray(Wqkv, np.float64), np.asarray(bqkv, np.float64)[None, :]], axis=0
    )
    Wq, Wk, Wv = waug[:, 0:E], waug[:, E : 2 * E], waug[:, 2 * E : 3 * E]
    Wff = np.asarray(Wff, np.float64)
    wp = np.zeros((EA, WPACK_COLS), np.float16)
    for h in range(H):
        hd = slice(h * D, (h + 1) * D)
        Ph = Wq[:, hd] @ Wk[:, hd].T                    # [97, 97]
        Qh = SCALE * (Wv[:, hd] @ Wff[hd, :])           # [97, 96]
        wp[0:EA, C_P + h * EA : C_P + (h + 1) * EA] = Ph.T.astype(np.float16)
        wp[0:EA, C_Q + h * E : C_Q + (h + 1) * E] = Qh.astype(np.float16)
    wp[0, C_OH + E] = 1.0                               # e_last selector row
    wp[0, C_BF : C_BF + E] = np.asarray(bff, np.float16)
    return {"wpack": wp}


def make_in_maps(x, Wqkv, bqkv, Wff, bff):
    x = np.asarray(x, np.float32)
    w = _host_weights(Wqkv, bqkv, Wff, bff)
    ones = np.ones((N, 1), np.float16)
    x16 = x.astype(np.float16)
    in_maps = []
    for c in range(N_CORES):
        b, h = divmod(c, 2)
        xb = x16[b]
        if h:
            xb = np.concatenate([xb[N // 2 :], xb[: N // 2]], axis=0)
        m = {"x": np.ascontiguousarray(np.concatenate([xb, ones], axis=1))}
        m.update(w)
        in_maps.append(m)
    return in_maps


def assemble(results):
    out = np.empty((B, N, E), np.float32)
    for c in range(N_CORES):
        b, h = divmod(c, 2)
        out[b, h * (N // 2) : (h + 1) * (N // 2)] = results[c]["out"]
    return out


def kernel(x, Wqkv, bqkv, Wff, bff):
    global LAST_RESULTS
    nc = get_nc()
    in_maps = make_in_maps(x, Wqkv, bqkv, Wff, bff)
    res = bass_utils.run_bass_kernel_spmd(
        nc, in_maps, core_ids=list(range(N_CORES))
    )
    LAST_RESULTS = res
    return assemble(res.results)

